# revision 2
# baseline (speedup 1.0000x reference)
"""Trainium2 Bass kernel for nn_BCNet: three-way low-rank bilinear net.

reference:
  v_ = relu(v @ Wv.T + bv)            # (B, NV, HK)
  q_ = relu(q @ Wq.T + bq)            # (B, NQ, HK)
  logits = einsum('hk,bvk,bqk->bhvq', h_mat, v_, q_) + h_bias

Sharding: data-parallel over batch, 4 batch items per core (8 cores).
All matmuls in bf16 with fp32 PSUM accumulation.

Host prep per core:
  vT   (4, 2048, 512) bf16  : v[b].T per batch item
  qT   (1024, 512)    bf16  : q[4c:4c+4] transposed+stacked, cols = b*128+q
  WvT  (2048, 1536)   bf16
  WqT  (1024, 1536)   bf16
  bvT  (128, 12) f32 : bv[jc*128+p]
  bqT  (128, 12) f32
  hm   (128, 12, 8) f32 : h_mat[h, jc*128+p]
  hb   (128, 1024) f32 : h_bias[col // 128] broadcast over partitions
Device output per core: out (4, 512, 1024) fp16, cols = h*128+q.
Host post: concat -> f32 -> (32, 512, 8, 128) -> transpose -> (32, 8, 512, 128).

DMA plan: sync issues qT + WqT (WqT split per (jg,d) so arrival matches
stage B's consumption order); gpsimd issues WvT + vT; scalar issues the
small constants and all output stores. A few warm-up matmuls on a memset
scratch tile run during the initial DMA wait so the PE p-state ramp is
done before real work starts.
"""

import numpy as np

B, NV, NQ = 32, 512, 128
V_DIM, Q_DIM, HK, H_OUT = 2048, 1024, 1536, 8
N_CORES = 8
BPC = B // N_CORES          # 4 batch items per core
JC = HK // 128              # 12 k-chunks
DCV = V_DIM // 128          # 16 contraction chunks for v
DCQ = Q_DIM // 128          # 8 contraction chunks for q
VC = NV // 128              # 4 v-chunks

_CACHE = {}


def _build_nc():
    import concourse.tile as tile
    from concourse import bacc, mybir
    from contextlib import ExitStack

    bf16 = mybir.dt.bfloat16
    f16 = mybir.dt.float16
    f32 = mybir.dt.float32

    nc = bacc.Bacc()

    vT = nc.declare_dram_parameter("vT", [BPC, V_DIM, NV], bf16, isOutput=False)
    qT = nc.declare_dram_parameter("qT", [Q_DIM, BPC * NQ], bf16, isOutput=False)
    WvT = nc.declare_dram_parameter("WvT", [V_DIM, HK], bf16, isOutput=False)
    WqT = nc.declare_dram_parameter("WqT", [Q_DIM, HK], bf16, isOutput=False)
    bvT = nc.declare_dram_parameter("bvT", [128, JC], f32, isOutput=False)
    bqT = nc.declare_dram_parameter("bqT", [128, JC], f32, isOutput=False)
    hm = nc.declare_dram_parameter("hm", [128, JC, H_OUT], f32, isOutput=False)
    hb = nc.declare_dram_parameter("hb", [128, H_OUT * NQ], f32, isOutput=False)
    out = nc.declare_dram_parameter("out", [BPC, NV, H_OUT * NQ], f16, isOutput=True)

    with ExitStack() as ctx:
        tc = ctx.enter_context(tile.TileContext(nc))
        consts = ctx.enter_context(tc.tile_pool(name="consts", bufs=1))
        qpool = ctx.enter_context(tc.tile_pool(name="qpool", bufs=1))
        vin = ctx.enter_context(tc.tile_pool(name="vin", bufs=2))
        vact = ctx.enter_context(tc.tile_pool(name="vact", bufs=2))
        qhp = ctx.enter_context(tc.tile_pool(name="qhp", bufs=1))
        outp = ctx.enter_context(tc.tile_pool(name="outp", bufs=3))
        warmp = ctx.enter_context(tc.tile_pool(name="warmp", bufs=1))
        psAB = ctx.enter_context(tc.tile_pool(name="psAB", bufs=4, space="PSUM"))
        psC = ctx.enter_context(tc.tile_pool(name="psC", bufs=4, space="PSUM"))

        # ---- input DMAs, split across issue engines -------------------
        # sync: qT chunks then WqT split per (jg, d) in stage B's exact
        # consumption order, so the first matmul only waits for 2 chunks.
        qT_r = qT.rearrange("(d p) n -> p d n", p=128)
        qt_sb = qpool.tile([128, DCQ, BPC * NQ], bf16)
        for dd in range(DCQ):
            nc.sync.dma_start(out=qt_sb[:, dd, :], in_=qT_r[:, dd, :])
        WqT_r = WqT.rearrange("(d p) j -> p d j", p=128)
        wq_sb = consts.tile([128, DCQ, HK], bf16)
        for jg in range(0, JC, 4):
            cs = slice(jg * 128, (jg + 4) * 128)
            for dd in range(DCQ):
                nc.sync.dma_start(out=wq_sb[:, dd, cs], in_=WqT_r[:, dd, cs])

        # scalar: small constants (needed from ~19us on), then per-vc
        # output stores later.
        bq_sb = consts.tile([128, JC], f32)
        nc.scalar.dma_start(out=bq_sb, in_=bqT[:, :])
        hm_sb = consts.tile([128, JC, H_OUT], f32)
        nc.scalar.dma_start(out=hm_sb, in_=hm[:, :, :])
        bv_sb = consts.tile([128, JC], f32)
        nc.scalar.dma_start(out=bv_sb, in_=bvT[:, :])
        hb_sb = consts.tile([128, H_OUT * NQ], f32)
        nc.scalar.dma_start(out=hb_sb, in_=hb[:, :])

        # gpsimd: WvT interleaved with b=0's vT chunks (stage A consumes
        # (wv[d], vt[d]) pairs in order), then per-b vT prefetches.
        WvT_r = WvT.rearrange("(d p) j -> p d j", p=128)
        wv_sb = consts.tile([128, DCV, HK], bf16)
        vt0_sb = vin.tile([128, DCV, NV], bf16, tag="vt", name="vt0")
        vT0_r = vT[0].rearrange("(d p) n -> p d n", p=128)
        for dd in range(DCV):
            nc.gpsimd.dma_start(out=wv_sb[:, dd, :], in_=WvT_r[:, dd, :])
            nc.gpsimd.dma_start(out=vt0_sb[:, dd, :], in_=vT0_r[:, dd, :])

        # ---- PE warm-up: ramp the p-state while first DMAs land -------
        warm_sb = warmp.tile([128, 512], bf16)
        nc.vector.memset(warm_sb, 0.0)
        warm_ps = psC.tile([128, 512], f32, tag="psC", name="warm")
        for i in range(4):
            nc.tensor.matmul(
                warm_ps,
                lhsT=warm_sb[:, 0:128],
                rhs=warm_sb[:, :],
                start=(i == 0),
                stop=(i == 3),
            )

        # ---- stage B: q_ = relu(q @ Wq.T + bq), all 4 b at once ----
        # d-outer within groups of 4 j's: weight chunk (jg, d) is consumed
        # right after its DMA lands.
        qact_sb = qpool.tile([128, JC, BPC * NQ], bf16)
        for jg in range(0, JC, 4):
            pss = [psAB.tile([128, BPC * NQ], f32, tag="psAB", name=f"psB{jg}_{i}") for i in range(4)]
            for d in range(DCQ):
                for ji in range(4):
                    j = jg + ji
                    nc.tensor.matmul(
                        pss[ji],
                        lhsT=wq_sb[:, d, j * 128:(j + 1) * 128],
                        rhs=qt_sb[:, d, :],
                        start=(d == 0),
                        stop=(d == DCQ - 1),
                    )
            for ji in range(4):
                j = jg + ji
                nc.scalar.activation(
                    out=qact_sb[:, j, :],
                    in_=pss[ji],
                    func=mybir.ActivationFunctionType.Relu,
                    bias=bq_sb[:, j:j + 1],
                    scale=1.0,
                )

        for b in range(BPC):
            # ---- build Qh[b][k, h*128+q'] = q_[k, b*128+q'] * h_mat[h, k] (DVE)
            qh_sb = qhp.tile([128, JC, H_OUT * NQ], bf16, tag="qh")
            for j in range(JC):
                for h in range(H_OUT):
                    nc.vector.tensor_scalar_mul(
                        qh_sb[:, j, h * NQ:(h + 1) * NQ],
                        qact_sb[:, j, b * NQ:(b + 1) * NQ],
                        hm_sb[:, j, h:h + 1],
                    )

            # ---- stage A: v_[b] = relu(v[b] @ Wv.T + bv), transposed layout
            if b == 0:
                vt_sb = vt0_sb
            else:
                vt_sb = vin.tile([128, DCV, NV], bf16, tag="vt")
                vT_r = vT[b].rearrange("(d p) n -> p d n", p=128)
                for dd in range(DCV):
                    nc.gpsimd.dma_start(out=vt_sb[:, dd, :], in_=vT_r[:, dd, :])
            vact_sb = vact.tile([128, JC, NV], bf16, tag="vact")
            for jg in range(0, JC, 4):
                pss = [psAB.tile([128, NV], f32, tag="psAB", name=f"psA{b}_{jg}_{i}") for i in range(4)]
                for d in range(DCV):
                    for ji in range(4):
                        j = jg + ji
                        nc.tensor.matmul(
                            pss[ji],
                            lhsT=wv_sb[:, d, j * 128:(j + 1) * 128],
                            rhs=vt_sb[:, d, :],
                            start=(d == 0),
                            stop=(d == DCV - 1),
                        )
                for ji in range(4):
                    j = jg + ji
                    nc.scalar.activation(
                        out=vact_sb[:, j, :],
                        in_=pss[ji],
                        func=mybir.ActivationFunctionType.Relu,
                        bias=bv_sb[:, j:j + 1],
                        scale=1.0,
                    )

            # ---- stage C: logits[b] = v_[b] @ Qh[b] (contract over k)
            # per-bank PSUM tiles: the add for half 0 runs while half 1's
            # matmuls are still accumulating.
            for vc in range(VC):
                po = [psC.tile([128, 512], f32, tag="psC", name=f"po{b}_{vc}_{i}") for i in range(2)]
                o_sb = outp.tile([128, H_OUT * NQ], f16, tag="osb")
                for nh in range(2):
                    for j in range(JC):
                        nc.tensor.matmul(
                            po[nh],
                            lhsT=vact_sb[:, j, vc * 128:(vc + 1) * 128],
                            rhs=qh_sb[:, j, nh * 512:(nh + 1) * 512],
                            start=(j == 0),
                            stop=(j == JC - 1),
                        )
                    sl = slice(nh * 512, (nh + 1) * 512)
                    nc.vector.tensor_add(o_sb[:, sl], po[nh], hb_sb[:, sl])
                nc.scalar.dma_start(
                    out=out[b, vc * 128:(vc + 1) * 128, :], in_=o_sb
                )

    nc.compile()
    return nc


def kernel(v, q, Wv, bv, Wq, bq, h_mat, h_bias):
    import ml_dtypes
    from concourse import bass_utils

    bf16 = ml_dtypes.bfloat16

    if "nc" not in _CACHE:
        _CACHE["nc"] = _build_nc()
    nc = _CACHE["nc"]

    v = np.asarray(v, dtype=np.float32)
    q = np.asarray(q, dtype=np.float32)
    Wv = np.asarray(Wv, dtype=np.float32)
    Wq = np.asarray(Wq, dtype=np.float32)
    bv = np.asarray(bv, dtype=np.float32)
    bq = np.asarray(bq, dtype=np.float32)
    h_mat = np.asarray(h_mat, dtype=np.float32)
    h_bias = np.asarray(h_bias, dtype=np.float32)

    vT = np.ascontiguousarray(v.transpose(0, 2, 1)).astype(bf16)      # (B, 2048, 512)
    WvT = np.ascontiguousarray(Wv.T).astype(bf16)                     # (2048, 1536)
    WqT = np.ascontiguousarray(Wq.T).astype(bf16)                     # (1024, 1536)
    bvT = np.ascontiguousarray(bv.reshape(JC, 128).T)                 # (128, 12)
    bqT = np.ascontiguousarray(bq.reshape(JC, 128).T)
    # hm[p, jc, h] = h_mat[h, jc*128+p]
    hmP = np.ascontiguousarray(h_mat.reshape(H_OUT, JC, 128).transpose(2, 1, 0))
    hbB = np.ascontiguousarray(
        np.broadcast_to(np.repeat(h_bias, NQ)[None, :], (128, H_OUT * NQ))
    )

    in_maps = []
    for c in range(N_CORES):
        bs = slice(BPC * c, BPC * (c + 1))
        qTc = np.ascontiguousarray(
            q[bs].transpose(2, 0, 1).reshape(Q_DIM, BPC * NQ)
        ).astype(bf16)
        in_maps.append({
            "vT": vT[bs],
            "qT": qTc,
            "WvT": WvT,
            "WqT": WqT,
            "bvT": bvT,
            "bqT": bqT,
            "hm": hmP,
            "hb": hbB,
        })

    res = bass_utils.run_bass_kernel_spmd(nc, in_maps, list(range(N_CORES)))
    outs = np.concatenate(
        [res.results[c]["out"].astype(np.float32) for c in range(N_CORES)], axis=0
    )
    # (32, 512, 1024) -> (32, 512, 8, 128) -> (32, 8, 512, 128)
    logits = outs.reshape(B, NV, H_OUT, NQ).transpose(0, 2, 1, 3)
    return np.ascontiguousarray(logits)


# revision 4
# speedup vs baseline: 1.0452x; 1.0452x over previous
"""Trainium2 Bass kernel for nn_BCNet: three-way low-rank bilinear net.

reference:
  v_ = relu(v @ Wv.T + bv)            # (B, NV, HK)
  q_ = relu(q @ Wq.T + bq)            # (B, NQ, HK)
  logits = einsum('hk,bvk,bqk->bhvq', h_mat, v_, q_) + h_bias

Sharding: data-parallel over batch, 4 batch items per core (8 cores).
All matmuls in bf16 with fp32 PSUM accumulation.

Host prep per core:
  vT   (4, 2048, 512) bf16  : v[b].T per batch item
  qT   (1024, 512)    bf16  : q[4c:4c+4] transposed+stacked, cols = b*128+q
  WvT  (2048, 1536)   bf16
  WqT  (1024, 1536)   bf16
  bvT  (128, 12) f32 : bv[jc*128+p]
  bqT  (128, 12) f32
  hm   (128, 12, 8) f32 : h_mat[h, jc*128+p]
  hb   (128, 1024) f32 : h_bias[col // 128] broadcast over partitions
Device output per core: out (4, 512, 1024) fp16, cols = h*128+q.
Host post: concat -> f32 -> (32, 512, 8, 128) -> transpose -> (32, 8, 512, 128).

DMA plan: sync issues qT + WqT (WqT split per (jg,d) so arrival matches
stage B's consumption order); gpsimd issues WvT + vT; scalar issues the
small constants and all output stores. A few warm-up matmuls on a memset
scratch tile run during the initial DMA wait so the PE p-state ramp is
done before real work starts.
"""

import numpy as np

B, NV, NQ = 32, 512, 128
V_DIM, Q_DIM, HK, H_OUT = 2048, 1024, 1536, 8
N_CORES = 8
BPC = B // N_CORES          # 4 batch items per core
JC = HK // 128              # 12 k-chunks
DCV = V_DIM // 128          # 16 contraction chunks for v
DCQ = Q_DIM // 128          # 8 contraction chunks for q
VC = NV // 128              # 4 v-chunks

_CACHE = {}


def _build_nc():
    import concourse.tile as tile
    from concourse import bacc, mybir
    from contextlib import ExitStack

    bf16 = mybir.dt.bfloat16
    f16 = mybir.dt.float16
    f32 = mybir.dt.float32

    nc = bacc.Bacc()

    vT = nc.declare_dram_parameter("vT", [BPC, V_DIM, NV], bf16, isOutput=False)
    qT = nc.declare_dram_parameter("qT", [Q_DIM, BPC * NQ], bf16, isOutput=False)
    WvT = nc.declare_dram_parameter("WvT", [V_DIM, HK], bf16, isOutput=False)
    WqT = nc.declare_dram_parameter("WqT", [Q_DIM, HK], bf16, isOutput=False)
    bvT = nc.declare_dram_parameter("bvT", [128, JC], f32, isOutput=False)
    bqT = nc.declare_dram_parameter("bqT", [128, JC], f32, isOutput=False)
    hm = nc.declare_dram_parameter("hm", [128, JC, H_OUT], f32, isOutput=False)
    hb = nc.declare_dram_parameter("hb", [128, H_OUT * NQ], f32, isOutput=False)
    out = nc.declare_dram_parameter("out", [BPC, NV, H_OUT * NQ], f16, isOutput=True)

    with ExitStack() as ctx:
        tc = ctx.enter_context(tile.TileContext(nc))
        consts = ctx.enter_context(tc.tile_pool(name="consts", bufs=1))
        qpool = ctx.enter_context(tc.tile_pool(name="qpool", bufs=1))
        vin = ctx.enter_context(tc.tile_pool(name="vin", bufs=2))
        vact = ctx.enter_context(tc.tile_pool(name="vact", bufs=2))
        qhp = ctx.enter_context(tc.tile_pool(name="qhp", bufs=1))
        outp = ctx.enter_context(tc.tile_pool(name="outp", bufs=3))
        warmp = ctx.enter_context(tc.tile_pool(name="warmp", bufs=1))
        psAB = ctx.enter_context(tc.tile_pool(name="psAB", bufs=4, space="PSUM"))
        psC = ctx.enter_context(tc.tile_pool(name="psC", bufs=4, space="PSUM"))

        # ---- input DMAs ----------------------------------------------
        # All bulk inputs on the sync queue in EXACT consumption order:
        # bytes on the wire in the order the PE needs them. Issue cost is
        # ~0.6us each and the shared DMA-sem pool chains issues to earlier
        # completions, so a single in-order queue beats "parallel" issue.
        # Front: (qt[d], wq[jg0,d]) pairs so the first matmul only waits
        # for 2 small chunks.
        qT_r = qT.rearrange("(d p) n -> p d n", p=128)
        qt_sb = qpool.tile([128, DCQ, BPC * NQ], bf16)
        WqT_r = WqT.rearrange("(d p) j -> p d j", p=128)
        wq_sb = consts.tile([128, DCQ, HK], bf16)
        cs0 = slice(0, 512)
        for dd in range(DCQ):
            nc.sync.dma_start(out=qt_sb[:, dd, :], in_=qT_r[:, dd, :])
            nc.sync.dma_start(out=wq_sb[:, dd, cs0], in_=WqT_r[:, dd, cs0])
        for jg in range(4, JC, 4):
            cs = slice(jg * 128, (jg + 4) * 128)
            for dd in range(DCQ):
                nc.sync.dma_start(out=wq_sb[:, dd, cs], in_=WqT_r[:, dd, cs])

        # scalar queue: small constants (bq needed at first ACTIVATE
        # ~19us), then per-vc output stores later.
        bq_sb = consts.tile([128, JC], f32)
        nc.scalar.dma_start(out=bq_sb, in_=bqT[:, :])
        hm_sb = consts.tile([128, JC, H_OUT], f32)
        nc.scalar.dma_start(out=hm_sb, in_=hm[:, :, :])
        bv_sb = consts.tile([128, JC], f32)
        nc.scalar.dma_start(out=bv_sb, in_=bvT[:, :])
        hb_sb = consts.tile([128, H_OUT * NQ], f32)
        nc.scalar.dma_start(out=hb_sb, in_=hb[:, :])

        # WvT interleaved with b=0's vT chunks (stage A consumes
        # (wv[d], vt[d]) pairs in order), after WqT on the sync queue.
        WvT_r = WvT.rearrange("(d p) j -> p d j", p=128)
        wv_sb = consts.tile([128, DCV, HK], bf16)
        vt0_sb = vin.tile([128, DCV, NV], bf16, tag="vt", name="vt0")
        vT0_r = vT[0].rearrange("(d p) n -> p d n", p=128)
        for dd in range(DCV):
            nc.sync.dma_start(out=wv_sb[:, dd, :], in_=WvT_r[:, dd, :])
            nc.sync.dma_start(out=vt0_sb[:, dd, :], in_=vT0_r[:, dd, :])

        # ---- PE warm-up: ramp the p-state while first DMAs land -------
        warm_sb = warmp.tile([128, 512], bf16)
        nc.vector.memset(warm_sb, 0.0)
        warm_ps = psC.tile([128, 512], f32, tag="psC", name="warm")
        for i in range(4):
            nc.tensor.matmul(
                warm_ps,
                lhsT=warm_sb[:, 0:128],
                rhs=warm_sb[:, :],
                start=(i == 0),
                stop=(i == 3),
            )

        # ---- stage B: q_ = relu(q @ Wq.T + bq), all 4 b at once ----
        # d-outer within groups of 4 j's: weight chunk (jg, d) is consumed
        # right after its DMA lands.
        qact_sb = qpool.tile([128, JC, BPC * NQ], bf16)
        for jg in range(0, JC, 4):
            pss = [psAB.tile([128, BPC * NQ], f32, tag="psAB", name=f"psB{jg}_{i}") for i in range(4)]
            for d in range(DCQ):
                for ji in range(4):
                    j = jg + ji
                    nc.tensor.matmul(
                        pss[ji],
                        lhsT=wq_sb[:, d, j * 128:(j + 1) * 128],
                        rhs=qt_sb[:, d, :],
                        start=(d == 0),
                        stop=(d == DCQ - 1),
                    )
            for ji in range(4):
                j = jg + ji
                nc.scalar.activation(
                    out=qact_sb[:, j, :],
                    in_=pss[ji],
                    func=mybir.ActivationFunctionType.Relu,
                    bias=bq_sb[:, j:j + 1],
                    scale=1.0,
                )

        for b in range(BPC):
            # ---- build Qh[b][k, h*128+q'] = q_[k, b*128+q'] * h_mat[h, k] (DVE)
            qh_sb = qhp.tile([128, JC, H_OUT * NQ], bf16, tag="qh")
            for j in range(JC):
                for h in range(H_OUT):
                    nc.vector.tensor_scalar_mul(
                        qh_sb[:, j, h * NQ:(h + 1) * NQ],
                        qact_sb[:, j, b * NQ:(b + 1) * NQ],
                        hm_sb[:, j, h:h + 1],
                    )

            # ---- stage A: v_[b] = relu(v[b] @ Wv.T + bv), transposed layout
            if b == 0:
                vt_sb = vt0_sb
            else:
                vt_sb = vin.tile([128, DCV, NV], bf16, tag="vt")
                vT_r = vT[b].rearrange("(d p) n -> p d n", p=128)
                for dd in range(0, DCV, 4):
                    nc.sync.dma_start(
                        out=vt_sb[:, dd:dd + 4, :], in_=vT_r[:, dd:dd + 4, :]
                    )
            vact_sb = vact.tile([128, JC, NV], bf16, tag="vact")
            for jg in range(0, JC, 4):
                pss = [psAB.tile([128, NV], f32, tag="psAB", name=f"psA{b}_{jg}_{i}") for i in range(4)]
                for d in range(DCV):
                    for ji in range(4):
                        j = jg + ji
                        nc.tensor.matmul(
                            pss[ji],
                            lhsT=wv_sb[:, d, j * 128:(j + 1) * 128],
                            rhs=vt_sb[:, d, :],
                            start=(d == 0),
                            stop=(d == DCV - 1),
                        )
                for ji in range(4):
                    j = jg + ji
                    nc.scalar.activation(
                        out=vact_sb[:, j, :],
                        in_=pss[ji],
                        func=mybir.ActivationFunctionType.Relu,
                        bias=bv_sb[:, j:j + 1],
                        scale=1.0,
                    )

            # ---- stage C: logits[b] = v_[b] @ Qh[b] (contract over k)
            # per-bank PSUM tiles: the add for half 0 runs while half 1's
            # matmuls are still accumulating.
            for vc in range(VC):
                po = [psC.tile([128, 512], f32, tag="psC", name=f"po{b}_{vc}_{i}") for i in range(2)]
                o_sb = outp.tile([128, H_OUT * NQ], f16, tag="osb")
                for nh in range(2):
                    for j in range(JC):
                        nc.tensor.matmul(
                            po[nh],
                            lhsT=vact_sb[:, j, vc * 128:(vc + 1) * 128],
                            rhs=qh_sb[:, j, nh * 512:(nh + 1) * 512],
                            start=(j == 0),
                            stop=(j == JC - 1),
                        )
                    sl = slice(nh * 512, (nh + 1) * 512)
                    nc.vector.tensor_add(o_sb[:, sl], po[nh], hb_sb[:, sl])
                nc.scalar.dma_start(
                    out=out[b, vc * 128:(vc + 1) * 128, :], in_=o_sb
                )

    nc.compile()
    return nc


def kernel(v, q, Wv, bv, Wq, bq, h_mat, h_bias):
    import ml_dtypes
    from concourse import bass_utils

    bf16 = ml_dtypes.bfloat16

    if "nc" not in _CACHE:
        _CACHE["nc"] = _build_nc()
    nc = _CACHE["nc"]

    v = np.asarray(v, dtype=np.float32)
    q = np.asarray(q, dtype=np.float32)
    Wv = np.asarray(Wv, dtype=np.float32)
    Wq = np.asarray(Wq, dtype=np.float32)
    bv = np.asarray(bv, dtype=np.float32)
    bq = np.asarray(bq, dtype=np.float32)
    h_mat = np.asarray(h_mat, dtype=np.float32)
    h_bias = np.asarray(h_bias, dtype=np.float32)

    vT = np.ascontiguousarray(v.transpose(0, 2, 1)).astype(bf16)      # (B, 2048, 512)
    WvT = np.ascontiguousarray(Wv.T).astype(bf16)                     # (2048, 1536)
    WqT = np.ascontiguousarray(Wq.T).astype(bf16)                     # (1024, 1536)
    bvT = np.ascontiguousarray(bv.reshape(JC, 128).T)                 # (128, 12)
    bqT = np.ascontiguousarray(bq.reshape(JC, 128).T)
    # hm[p, jc, h] = h_mat[h, jc*128+p]
    hmP = np.ascontiguousarray(h_mat.reshape(H_OUT, JC, 128).transpose(2, 1, 0))
    hbB = np.ascontiguousarray(
        np.broadcast_to(np.repeat(h_bias, NQ)[None, :], (128, H_OUT * NQ))
    )

    in_maps = []
    for c in range(N_CORES):
        bs = slice(BPC * c, BPC * (c + 1))
        qTc = np.ascontiguousarray(
            q[bs].transpose(2, 0, 1).reshape(Q_DIM, BPC * NQ)
        ).astype(bf16)
        in_maps.append({
            "vT": vT[bs],
            "qT": qTc,
            "WvT": WvT,
            "WqT": WqT,
            "bvT": bvT,
            "bqT": bqT,
            "hm": hmP,
            "hb": hbB,
        })

    res = bass_utils.run_bass_kernel_spmd(nc, in_maps, list(range(N_CORES)))
    outs = np.concatenate(
        [res.results[c]["out"].astype(np.float32) for c in range(N_CORES)], axis=0
    )
    # (32, 512, 1024) -> (32, 512, 8, 128) -> (32, 8, 512, 128)
    logits = outs.reshape(B, NV, H_OUT, NQ).transpose(0, 2, 1, 3)
    return np.ascontiguousarray(logits)


# revision 8
# speedup vs baseline: 1.0823x; 1.0355x over previous
"""Trainium2 Bass kernel for nn_BCNet: three-way low-rank bilinear net.

reference:
  v_ = relu(v @ Wv.T + bv)            # (B, NV, HK)
  q_ = relu(q @ Wq.T + bq)            # (B, NQ, HK)
  logits = einsum('hk,bvk,bqk->bhvq', h_mat, v_, q_) + h_bias

Sharding: data-parallel over batch, 4 batch items per core (8 cores).
All matmuls in bf16 with fp32 PSUM accumulation.

Host prep per core:
  vT   (4, 2048, 512) bf16  : v[b].T per batch item
  qT   (1024, 512)    bf16  : q[4c:4c+4] transposed+stacked, cols = b*128+q
  WvT  (2048, 1536)   bf16
  WqT  (1024, 1536)   bf16
  bvT  (128, 12) f32 : bv[jc*128+p]
  bqT  (128, 12) f32
  hm   (128, 12, 8) f32 : h_mat[h, jc*128+p]
  hb   (128, 1024) f32 : h_bias[col // 128] broadcast over partitions
Device output per core: out (4, 512, 1024) fp16, cols = h*128+q.
Host post: concat -> f32 -> (32, 512, 8, 128) -> transpose -> (32, 8, 512, 128).

DMA plan: sync issues qT + WqT (WqT split per (jg,d) so arrival matches
stage B's consumption order); gpsimd issues WvT + vT; scalar issues the
small constants and all output stores. A few warm-up matmuls on a memset
scratch tile run during the initial DMA wait so the PE p-state ramp is
done before real work starts.
"""

import numpy as np

B, NV, NQ = 32, 512, 128
V_DIM, Q_DIM, HK, H_OUT = 2048, 1024, 1536, 8
N_CORES = 8
BPC = B // N_CORES          # 4 batch items per core
JC = HK // 128              # 12 k-chunks
DCV = V_DIM // 128          # 16 contraction chunks for v
DCQ = Q_DIM // 128          # 8 contraction chunks for q
VC = NV // 128              # 4 v-chunks

_CACHE = {}


def _build_nc():
    import concourse.tile as tile
    from concourse import bacc, mybir
    from contextlib import ExitStack

    bf16 = mybir.dt.bfloat16
    f16 = mybir.dt.float16
    f32 = mybir.dt.float32

    nc = bacc.Bacc()

    vT = nc.declare_dram_parameter("vT", [BPC, V_DIM, NV], bf16, isOutput=False)
    qT = nc.declare_dram_parameter("qT", [Q_DIM, BPC * NQ], bf16, isOutput=False)
    WvT = nc.declare_dram_parameter("WvT", [V_DIM, HK], bf16, isOutput=False)
    WqT = nc.declare_dram_parameter("WqT", [Q_DIM, HK], bf16, isOutput=False)
    bvT = nc.declare_dram_parameter("bvT", [128, JC], f32, isOutput=False)
    bqT = nc.declare_dram_parameter("bqT", [128, JC], f32, isOutput=False)
    hm = nc.declare_dram_parameter("hm", [128, JC, H_OUT], f32, isOutput=False)
    hb = nc.declare_dram_parameter("hb", [128, H_OUT * NQ], f32, isOutput=False)
    out = nc.declare_dram_parameter("out", [BPC, NV, H_OUT * NQ], f16, isOutput=True)

    with ExitStack() as ctx:
        tc = ctx.enter_context(tile.TileContext(nc))
        consts = ctx.enter_context(tc.tile_pool(name="consts", bufs=1))
        qpool = ctx.enter_context(tc.tile_pool(name="qpool", bufs=1))
        vin = ctx.enter_context(tc.tile_pool(name="vin", bufs=2))
        vact = ctx.enter_context(tc.tile_pool(name="vact", bufs=2))
        qhp = ctx.enter_context(tc.tile_pool(name="qhp", bufs=1))
        outp = ctx.enter_context(tc.tile_pool(name="outp", bufs=3))
        warmp = ctx.enter_context(tc.tile_pool(name="warmp", bufs=1))
        psAB = ctx.enter_context(tc.tile_pool(name="psAB", bufs=5, space="PSUM"))
        psC = ctx.enter_context(tc.tile_pool(name="psC", bufs=3, space="PSUM"))

        # ---- input DMAs ----------------------------------------------
        # All bulk inputs on the sync queue in EXACT consumption order:
        # bytes on the wire in the order the PE needs them. Issue cost is
        # ~0.6us each and the shared DMA-sem pool chains issues to earlier
        # completions, so a single in-order queue beats "parallel" issue.
        # Front: (qt[d], wq[jg0,d]) pairs so the first matmul only waits
        # for 2 small chunks.
        # Geometric chunk sizing: tiny first chunks (early PE start), then
        # growing chunks so the ~0.6us/issue rate never limits the feed.
        qT_r = qT.rearrange("(d p) n -> p d n", p=128)
        qt_sb = qpool.tile([128, DCQ, BPC * NQ], bf16)
        WqT_r = WqT.rearrange("(d p) j -> p d j", p=128)
        wq_sb = consts.tile([128, DCQ, HK], bf16)
        cs0 = slice(0, 512)
        for d0, d1 in ((0, 1), (1, 2), (2, 4), (4, 8)):
            nc.sync.dma_start(out=qt_sb[:, d0:d1, :], in_=qT_r[:, d0:d1, :])
            nc.sync.dma_start(out=wq_sb[:, d0:d1, cs0], in_=WqT_r[:, d0:d1, cs0])
        cs1 = slice(512, 1024)
        nc.sync.dma_start(out=wq_sb[:, 0:4, cs1], in_=WqT_r[:, 0:4, cs1])
        nc.sync.dma_start(out=wq_sb[:, 4:8, cs1], in_=WqT_r[:, 4:8, cs1])
        cs2 = slice(1024, 1536)
        nc.sync.dma_start(out=wq_sb[:, :, cs2], in_=WqT_r[:, :, cs2])

        # scalar queue: small constants (bq needed at first ACTIVATE
        # ~19us), then per-vc output stores later.
        bq_sb = consts.tile([128, JC], f32)
        nc.scalar.dma_start(out=bq_sb, in_=bqT[:, :])
        hm_sb = consts.tile([128, JC, H_OUT], f32)
        nc.scalar.dma_start(out=hm_sb, in_=hm[:, :, :])
        bv_sb = consts.tile([128, JC], f32)
        nc.scalar.dma_start(out=bv_sb, in_=bvT[:, :])
        hb_sb = consts.tile([128, H_OUT * NQ], f32)
        nc.scalar.dma_start(out=hb_sb, in_=hb[:, :])

        # WvT interleaved with b=0's vT chunks (stage A consumes
        # (wv[d], vt[d]) pairs in order), after WqT on the sync queue.
        WvT_r = WvT.rearrange("(d p) j -> p d j", p=128)
        wv_sb = consts.tile([128, DCV, HK], bf16)
        vt0_sb = vin.tile([128, DCV, NV], bf16, tag="vt", name="vt0")
        vT0_r = vT[0].rearrange("(d p) n -> p d n", p=128)
        for d0, d1 in ((0, 1), (1, 2), (2, 6), (6, 10), (10, 14), (14, 16)):
            nc.sync.dma_start(out=wv_sb[:, d0:d1, :], in_=WvT_r[:, d0:d1, :])
            nc.sync.dma_start(out=vt0_sb[:, d0:d1, :], in_=vT0_r[:, d0:d1, :])

        # ---- PE warm-up: ramp the p-state while first DMAs land -------
        warm_sb = warmp.tile([128, 512], bf16)
        nc.vector.memset(warm_sb, 0.0)
        warm_ps = psC.tile([128, 512], f32, tag="psC", name="warm")
        for i in range(8):
            nc.tensor.matmul(
                warm_ps,
                lhsT=warm_sb[:, 0:128],
                rhs=warm_sb[:, :],
                start=(i == 0),
                stop=(i == 7),
            )

        # ---- stage B: q_ = relu(q @ Wq.T + bq), all 4 b at once ----
        # d-outer within groups of 4 j's: weight chunk (jg, d) is consumed
        # right after its DMA lands.
        qact_sb = qpool.tile([128, JC, BPC * NQ], bf16)
        for jg in range(0, JC, 4):
            pss = [psAB.tile([128, BPC * NQ], f32, tag="psAB", name=f"psB{jg}_{i}") for i in range(4)]
            for d in range(DCQ):
                for ji in range(4):
                    j = jg + ji
                    nc.tensor.matmul(
                        pss[ji],
                        lhsT=wq_sb[:, d, j * 128:(j + 1) * 128],
                        rhs=qt_sb[:, d, :],
                        start=(d == 0),
                        stop=(d == DCQ - 1),
                    )
            for ji in range(4):
                j = jg + ji
                nc.scalar.activation(
                    out=qact_sb[:, j, :],
                    in_=pss[ji],
                    func=mybir.ActivationFunctionType.Relu,
                    bias=bq_sb[:, j:j + 1],
                    scale=1.0,
                )

        for b in range(BPC):
            # ---- build Qh[b][k, h*128+q'] = q_[k, b*128+q'] * h_mat[h, k] (DVE)
            qh_sb = qhp.tile([128, JC, H_OUT * NQ], bf16, tag="qh")
            for j in range(JC):
                for h in range(H_OUT):
                    nc.vector.tensor_scalar_mul(
                        qh_sb[:, j, h * NQ:(h + 1) * NQ],
                        qact_sb[:, j, b * NQ:(b + 1) * NQ],
                        hm_sb[:, j, h:h + 1],
                    )

            # ---- stage A: v_[b] = relu(v[b] @ Wv.T + bv), transposed layout
            if b == 0:
                vt_sb = vt0_sb
            else:
                vt_sb = vin.tile([128, DCV, NV], bf16, tag="vt")
                vT_r = vT[b].rearrange("(d p) n -> p d n", p=128)
                for dd in range(0, DCV, 4):
                    nc.sync.dma_start(
                        out=vt_sb[:, dd:dd + 4, :], in_=vT_r[:, dd:dd + 4, :]
                    )
            vact_sb = vact.tile([128, JC, NV], bf16, tag="vact")
            for jg in range(0, JC, 4):
                pss = [psAB.tile([128, NV], f32, tag="psAB", name=f"psA{b}_{jg}_{i}") for i in range(4)]
                for d in range(DCV):
                    for ji in range(4):
                        j = jg + ji
                        nc.tensor.matmul(
                            pss[ji],
                            lhsT=wv_sb[:, d, j * 128:(j + 1) * 128],
                            rhs=vt_sb[:, d, :],
                            start=(d == 0),
                            stop=(d == DCV - 1),
                        )
                for ji in range(4):
                    j = jg + ji
                    nc.scalar.activation(
                        out=vact_sb[:, j, :],
                        in_=pss[ji],
                        func=mybir.ActivationFunctionType.Relu,
                        bias=bv_sb[:, j:j + 1],
                        scale=1.0,
                    )

            # ---- stage C: logits[b] = v_[b] @ Qh[b] (contract over k)
            # per-bank PSUM tiles: the add for half 0 runs while half 1's
            # matmuls are still accumulating.
            for vc in range(VC):
                po = [psC.tile([128, 512], f32, tag="psC", name=f"po{b}_{vc}_{i}") for i in range(2)]
                o_sb = outp.tile([128, H_OUT * NQ], f16, tag="osb")
                for nh in range(2):
                    for j in range(JC):
                        nc.tensor.matmul(
                            po[nh],
                            lhsT=vact_sb[:, j, vc * 128:(vc + 1) * 128],
                            rhs=qh_sb[:, j, nh * 512:(nh + 1) * 512],
                            start=(j == 0),
                            stop=(j == JC - 1),
                        )
                    sl = slice(nh * 512, (nh + 1) * 512)
                    nc.vector.tensor_add(o_sb[:, sl], po[nh], hb_sb[:, sl])
                nc.scalar.dma_start(
                    out=out[b, vc * 128:(vc + 1) * 128, :], in_=o_sb
                )

    nc.compile()
    return nc


def kernel(v, q, Wv, bv, Wq, bq, h_mat, h_bias):
    import ml_dtypes
    from concourse import bass_utils

    bf16 = ml_dtypes.bfloat16

    if "nc" not in _CACHE:
        _CACHE["nc"] = _build_nc()
    nc = _CACHE["nc"]

    v = np.asarray(v, dtype=np.float32)
    q = np.asarray(q, dtype=np.float32)
    Wv = np.asarray(Wv, dtype=np.float32)
    Wq = np.asarray(Wq, dtype=np.float32)
    bv = np.asarray(bv, dtype=np.float32)
    bq = np.asarray(bq, dtype=np.float32)
    h_mat = np.asarray(h_mat, dtype=np.float32)
    h_bias = np.asarray(h_bias, dtype=np.float32)

    vT = np.ascontiguousarray(v.transpose(0, 2, 1)).astype(bf16)      # (B, 2048, 512)
    WvT = np.ascontiguousarray(Wv.T).astype(bf16)                     # (2048, 1536)
    WqT = np.ascontiguousarray(Wq.T).astype(bf16)                     # (1024, 1536)
    bvT = np.ascontiguousarray(bv.reshape(JC, 128).T)                 # (128, 12)
    bqT = np.ascontiguousarray(bq.reshape(JC, 128).T)
    # hm[p, jc, h] = h_mat[h, jc*128+p]
    hmP = np.ascontiguousarray(h_mat.reshape(H_OUT, JC, 128).transpose(2, 1, 0))
    hbB = np.ascontiguousarray(
        np.broadcast_to(np.repeat(h_bias, NQ)[None, :], (128, H_OUT * NQ))
    )

    in_maps = []
    for c in range(N_CORES):
        bs = slice(BPC * c, BPC * (c + 1))
        qTc = np.ascontiguousarray(
            q[bs].transpose(2, 0, 1).reshape(Q_DIM, BPC * NQ)
        ).astype(bf16)
        in_maps.append({
            "vT": vT[bs],
            "qT": qTc,
            "WvT": WvT,
            "WqT": WqT,
            "bvT": bvT,
            "bqT": bqT,
            "hm": hmP,
            "hb": hbB,
        })

    res = bass_utils.run_bass_kernel_spmd(nc, in_maps, list(range(N_CORES)))
    outs = np.concatenate(
        [res.results[c]["out"].astype(np.float32) for c in range(N_CORES)], axis=0
    )
    # (32, 512, 1024) -> (32, 512, 8, 128) -> (32, 8, 512, 128)
    logits = outs.reshape(B, NV, H_OUT, NQ).transpose(0, 2, 1, 3)
    return np.ascontiguousarray(logits)
